# revision 34
# baseline (speedup 1.0000x reference)
"""Trainium2 Bass kernel: TextCNN (conv k=3/4/5 over [B,1,S,E] + relu +
global max-pool + FC + log_softmax), data-parallel over batch on 8 cores.

v2: conv contraction in fp8 e4m3 DoubleRow matmuls — each matmul
contracts 256 rows (two 128-row chunks paired along the DR j-dim), so a
group of 4 batch elems needs 15 matmuls instead of the fp32r version's
30, and fp8 streams at the PE's double rate. Weights are pre-scaled by
2^7 on the host to sit in e4m3's normal range; the scale is divided out
for free in the ReLU step via the activation unit's `scale` operand.
The E=300 contraction per tap splits as e[0:128]+e[128:256] paired in
one DR matmul (j = chunk index, identical column shift), plus a per-
branch host-baked tail matmul packing all taps' e[256:300] rows two-
per-partition. x for each 4-batch group lands in one [128, 4224B] DMA
(main pair block + 3 tail blocks); all 16 group DMAs are issued
upfront and stay resident in SBUF. FC + log_softmax stay fp32.

Every instruction carries <=1 semaphore wait (single wait slot in this
toolchain's TPB encodings): group g's x-DMA wait is pre-satisfied on
the last matmul of group g-1, so each branch-start matmul carries only
its PSUM-rotation wait. The kernel-tail drain is split per semaphore
proc.

Self-contained: hardcodes shapes/sharding; only imports the container
toolchain at /opt/trn_rl_repo.
"""

import sys

import ml_dtypes
import numpy as np

sys.path.insert(0, "/opt/trn_rl_repo")

import concourse.bass as bass  # noqa: E402
import concourse.tile as tile  # noqa: E402
from concourse import mybir  # noqa: E402
from concourse.bass_utils import run_bass_kernel_spmd  # noqa: E402
from concourse.tile import add_dep_helper  # noqa: E402
from concourse.vector_clock import ScopedClock, VectorClock  # noqa: E402

B, S, E = 512, 128, 300
NF = 100
NCLS = 5
NCORES = 8
BPC = B // NCORES  # 64 batch elems per core
G = 4  # batch elems per matmul group (4*128 = 512 psum cols)
NG = BPC // G  # 16 groups
PAD = 2
SP = S + 2 * PAD  # 132 padded seq length
KS = (3, 4, 5)
SOUT = {3: S - 2, 4: S - 1, 5: S}  # valid conv output positions per branch
SMM = S  # uniform matmul col window; invalid tail cols excluded by reduce
E2 = 256  # main contraction rows (two 128 chunks paired along DR j)
E2N = E - E2  # 44 tail rows per tap
WS = 128.0  # host weight scale into e4m3 range; divided out in ReLU
NFP = 112  # M padded to a 16B multiple (dual-fp8 LDW stride alignment)
# main taps in matmul order: (k, i, off); off = (5-k)+i is the xpad shift
MAINTAPS = [(k, i, 5 - k + i) for k in KS for i in range(k)]
NTAP = len(MAINTAPS)  # 12
NW = NTAP + 3  # + one packed tail per branch
WCOLS = NW * 2 * NFP  # 3360 fp8 weight cols
NMM = SMM * G  # 512 moving cols per matmul, s-major (n = s*G + b)
SPW = SP * G  # 528-col s-major main plane; shift o = slice [4o : 4o+512]
GCOLS = 2 * (SPW + 3 * NMM)  # 4128 fp8 x cols per group (j-major)
TOTC = WCOLS + NG * GCOLS
AUXW = 3 + 3 * NCLS
NPRE = 3  # prewarm matmuls bridging DMA ramp + PE clock ramp

_f32 = mybir.dt.float32
_fp8 = mybir.dt.float8e4
_DR = mybir.MatmulPerfMode.DoubleRow

_built = None


def _ins(i):
    return i.ins if hasattr(i, "ins") else i


def _dep(from_inst, to_inst, reason, sync=True):
    add_dep_helper(_ins(from_inst), _ins(to_inst), sync=sync, reason=reason)


class _SplitDrainTC(tile.TileContext):
    """TileContext whose kernel-tail drain is split into one drain per
    semaphore proc: the stock single drain carries one wait per used proc,
    which overflows the CTRL_NO encoding's wait slots on this toolchain."""

    def _drain_and_barrier(self, tick_clock, wait_clock):
        gc = tick_clock.global_clock
        ticks = eval(str(gc).replace("VectorClock", ""))
        for idx, tick in enumerate(ticks):
            if tick > 0:
                sub = VectorClock()
                sub.require_at_least(idx, tick)
                d = self.nc.sync.drain()
                wait_clock.add_sem_waits(d.ins, ScopedClock({None: sub}))
        self.nc.all_engine_barrier()
        assert self.sems is not None
        popped = self.nc._tile_sem_poison_stack.pop()
        assert popped is self._sem_poison
        self.nc.clear_and_free_semaphores(list(self.sems.allocated().values()))
        self.nc.all_engine_barrier()


def _build():
    nc = bass.Bass()
    xq = nc.declare_dram_parameter("xq", [128, TOTC], _fp8, isOutput=False)
    aux = nc.declare_dram_parameter("aux", [NF + 1, AUXW], _f32, isOutput=False)
    out = nc.declare_dram_parameter("out", [NCLS, BPC], _f32, isOutput=True)

    act = mybir.ActivationFunctionType

    with _SplitDrainTC(nc) as tc:
        with (
            tc.tile_pool(name="consts", bufs=1) as consts,
            tc.tile_pool(name="xin", bufs=NG) as xin,
            tc.tile_pool(name="small", bufs=4) as small,
            tc.tile_pool(name="feat", bufs=1) as featp,
            tc.tile_pool(name="psum", bufs=2, space="PSUM") as psum,
            tc.tile_pool(name="psfc", bufs=1, space="PSUM") as psfc,
        ):
            pescr = psfc.tile([128, 512], _f32, tag="pescr")
            junk = small.tile([128, 2, 512], _fp8, tag="junk")
            nc.vector.memset(junk[:], 0.25)

            # DMAs: weights first, then x groups (w, x0, x1, aux, x2..x15)
            wtile = consts.tile([128, NW, 2, NFP], _fp8, tag="w", name="w")
            wdma = nc.sync.dma_start(
                out=wtile[:],
                in_=xq[:, :WCOLS].rearrange("p (t j f) -> p t j f", t=NW, j=2),
            )

            xtiles = {}
            xmdmas = {}
            xtdmas = {}

            def make_x(g, eng=None):
                if g in xtiles:
                    return xtiles[g]
                t = xin.tile([128, 2, SPW + 3 * NMM], _fp8, tag="x", name=f"x_{g}")
                base = WCOLS + g * GCOLS
                eng = eng or nc.sync
                # main plane first as its own small DMA: it unblocks the
                # group's k=3 taps ~4x sooner than the full-group transfer
                # (the DMA engines run slow while ramping at kernel start)
                xmdmas[g] = eng.dma_start(
                    out=t[:, :, :SPW],
                    in_=xq[:, base : base + 2 * SPW].rearrange(
                        "p (j n) -> p j n", j=2
                    ),
                )
                xtdmas[g] = eng.dma_start(
                    out=t[:, :, SPW:],
                    in_=xq[:, base + 2 * SPW : base + GCOLS].rearrange(
                        "p (j n) -> p j n", j=2
                    ),
                )
                xtiles[g] = t
                return t

            # two DMA rings share HBM bandwidth: alternate groups across
            # them in need-order so the earliest groups land first (a ring
            # prefetching late groups would starve the critical early ones)
            make_x(0, eng=nc.gpsimd)
            make_x(1, eng=nc.sync)
            auxt = consts.tile([NF + 1, AUXW], _f32, tag="aux", name="aux")
            aux_dma = nc.gpsimd.dma_start(out=auxt[:], in_=aux[:, :])
            for g in range(2, NG):
                make_x(g, eng=nc.gpsimd if g % 2 == 0 else nc.sync)

            # prewarm: fp8 DR junk matmuls bridge the DMA ramp so the PE
            # clock is up when the real stream starts; last one fences wdma.
            # sync=False edges pin the scheduler to this PE order — without
            # them it hoists later matmuls above the wait-carrying ones and
            # the single-wait-slot budget breaks.
            last_pe = None
            for p in range(NPRE):
                pw = nc.tensor.matmul(
                    pescr[:, :],
                    junk[:, :, :128],
                    junk[:, :, :],
                    start=True,
                    stop=True,
                    perf_mode=_DR,
                )
                if last_pe is not None:
                    _dep(pw, last_pe, "pe chain", sync=False)
                last_pe = pw
                if p == NPRE - 1:
                    _dep(pw, wdma, "w loaded")

            ascratch = small.tile([1, 1], _f32, tag="ascratch")
            feats = [
                featp.tile([NF, BPC], _f32, tag=f"feat{kk}", name=f"feat{kk}")
                for kk in range(3)
            ]
            featr = [
                featp.tile(
                    [NF + (1 if kk == 2 else 0), BPC],
                    _f32,
                    tag=f"featr{kk}",
                    name=f"featr{kk}",
                )
                for kk in range(3)
            ]
            nc.vector.memset(featr[2][:], 1.0)

            plT = psfc.tile([NCLS, BPC], _f32, tag="plT")
            ones5 = small.tile([NCLS, 1], _f32, tag="ones5")
            nc.vector.memset(ones5[:], 1.0)
            mones1 = small.tile([1, NCLS], _f32, tag="mones1")
            nc.vector.memset(mones1[:], -1.0)
            afence = nc.scalar.memzero(ascratch[:])
            _dep(afence, aux_dma, "act waits aux")
            # touch Exp/Ln tables now so the tail doesn't pay cold loads
            nc.scalar.activation(ascratch[:], ascratch[:], act.Exp)
            nc.scalar.activation(ascratch[:], ascratch[:], act.Ln)

            gmms = {}  # group -> list of its 15 conv matmuls
            greds = {}  # group -> last reduce_max
            for g in range(NG):
                xt = xtiles[g]
                if g >= 1:
                    # pre-satisfy group g's cross-queue waits on spare
                    # (waitless) matmuls of group g-1, so this group's
                    # branch-start matmuls carry no >1-wait encodings:
                    #   x DMAs done, g-2's reduces done (frees PSUM banks,
                    #   DVE sem), g-2's stop-matmul completed (PE sem).
                    prev = gmms[g - 1]
                    _dep(prev[4], xmdmas[g], "x main presat")
                    _dep(prev[5], xtdmas[g], "x tail presat")
                    if g >= 2:
                        _dep(prev[8], greds[g - 2], "psum reduce presat")
                        _dep(prev[9], gmms[g - 2][14], "psum group presat")

                ti = 0
                mms = []

                def _mm(*args, **kw):
                    nonlocal last_pe
                    m = nc.tensor.matmul(*args, **kw)
                    _dep(m, last_pe, "pe chain", sync=False)
                    last_pe = m
                    mms.append(m)
                    return m

                def _fc(s, last):
                    for kk in range(3):
                        krows = NF + (1 if kk == 2 else 0)
                        wsl = auxt[:krows, 3 + NCLS * kk : 3 + NCLS * (kk + 1)]
                        fc = _mm(
                            plT[:, 16 * s : 16 * (s + 1)],
                            wsl,
                            featr[kk][:krows, 16 * s : 16 * (s + 1)],
                            start=(s == 0 and kk == 0),
                            stop=False,
                        )
                    return fc

                # FC block s runs two groups after its features are done, so
                # its Scalar-side relu wait is long satisfied (no PE stall)
                if g >= 5 and (g - 5) % 4 == 0:
                    _fc((g - 5) // 4, last=False)

                # main taps of all branches first (they need only the small
                # main-plane DMA), then the three tails — so the early
                # groups start well before their tail transfer lands. The
                # LAST group instead runs branch-sequential, so k3/k4's
                # reduce+relu+FC overlap the remaining conv matmuls instead
                # of serializing on the DVE after the final matmul.
                seq = g == NG - 1

                def _tail(kk, k, ps):
                    _mm(
                        ps[:, :, :],
                        wtile[:, NTAP + kk, :, :],
                        xt[:, :, SPW + kk * NMM : SPW + (kk + 1) * NMM],
                        start=False,
                        stop=True,
                        perf_mode=_DR,
                    )
                    red = nc.vector.reduce_max(
                        feats[kk][:, g * G : (g + 1) * G],
                        ps[:NF, : SOUT[k], :].transpose([0, 2, 1]),
                        axis=mybir.AxisListType.X,
                    )
                    r = nc.scalar.activation(
                        featr[kk][:NF, g * G : (g + 1) * G],
                        feats[kk][:, g * G : (g + 1) * G],
                        act.Relu,
                        bias=auxt[:NF, kk : kk + 1],
                        scale=1.0 / WS,
                    )
                    _dep(r, afence, "act fence", sync=False)
                    return red

                pss = {}
                for kk, k in enumerate(KS):
                    ps = psum.tile([NFP, SMM, G], _f32, tag=f"y{k}", name=f"y{k}_{g}")
                    pss[kk] = ps
                    for i in range(k):
                        off = 5 - k + i
                        _mm(
                            ps[:, :, :],
                            wtile[:, ti, :, :],
                            xt[:, :, 4 * off : 4 * off + NMM],
                            start=(i == 0),
                            stop=False,
                            perf_mode=_DR,
                        )
                        ti += 1
                    if seq:
                        red = _tail(kk, k, ps)
                if g == 2:
                    _dep(mms[1], aux_dma, "aux presat for FC")
                if not seq:
                    for kk, k in enumerate(KS):
                        red = _tail(kk, k, pss[kk])
                # conv matmuls only (indices 0..14) — FC mms are appended
                # after, so the carrier indices stay conv taps
                gmms[g] = mms[-15:]
                greds[g] = red

            _fc(3, last=True)

            # log_softmax in transposed layout: out = x - ln(sum exp x); the
            # class-dim sum is a tiny ones-matmul, the broadcast subtract a
            # single DVE op reading plT (PSUM) directly
            expT = small.tile([NCLS, BPC], _f32, tag="expT")
            nc.scalar.activation(expT[:], plT[:], act.Exp)
            nc.tensor.matmul(
                pescr[0:1, 64:128], ones5[:], expT[:], start=True, stop=True
            )
            lns = small.tile([1, BPC], _f32, tag="lns")
            nc.scalar.activation(lns[:], pescr[0:1, 64:128], act.Ln)
            nc.tensor.matmul(plT[:], mones1[:], lns[:], start=False, stop=True)
            # copy + output DMA both on the Scalar queue: the DMA then needs
            # no data wait (queue order) — only its hardware-ring slot
            ot = small.tile([NCLS, BPC], _f32, tag="ot")
            cp = nc.scalar.activation(ot[:], plT[:], act.Copy)
            oscr = small.tile([1, 1], _f32, tag="oscr")
            f2 = nc.scalar.memzero(oscr[:])
            _dep(f2, cp, "ot ready")
            nc.scalar.dma_start(out=out[:, :], in_=ot[:])
    return nc


def _prep(x, w3, b3, w4, b4, w5, b5, Wfc, bfc):
    x = np.asarray(x, dtype=np.float32).reshape(B, S, E)
    ws = {3: np.asarray(w3, np.float32)[:, 0], 4: np.asarray(w4, np.float32)[:, 0],
          5: np.asarray(w5, np.float32)[:, 0]}  # [NF, k, E]

    # weights region (identical across cores), assembled fp32 then cast once
    wreg = np.zeros((128, NW, 2, NFP), np.float32)
    for t, (k, i, _off) in enumerate(MAINTAPS):
        for j in range(2):
            wreg[:, t, j, :NF] = WS * ws[k][:, i, j * 128 : (j + 1) * 128].T
    for r, k in enumerate(KS):
        L = np.arange(E2N * k)
        i_of = L // E2N
        e_of = E2 + (L % E2N)
        wt = ws[k][:, i_of, e_of].T * WS  # [L, NF]
        wreg[L // 2, NTAP + r, L % 2, :NF] = wt
    wreg = wreg.reshape(128, WCOLS)

    # x padded + transposed: [E, B, SP]
    xt_all = np.zeros((E, B, SP), np.float32)
    xt_all[:, :, PAD : PAD + S] = x.transpose(2, 0, 1)

    auxm = np.zeros((NF + 1, AUXW), np.float32)
    for kk, bb in enumerate((b3, b4, b5)):
        auxm[:NF, kk] = np.asarray(bb, np.float32)
    Wfc = np.asarray(Wfc, np.float32)
    for kk in range(3):
        auxm[:NF, 3 + NCLS * kk : 3 + NCLS * (kk + 1)] = Wfc[
            :, kk * NF : (kk + 1) * NF
        ].T
    auxm[NF, 3 + 2 * NCLS : 3 + 3 * NCLS] = np.asarray(bfc, np.float32)

    shards = []
    for c in range(NCORES):
        arr = np.zeros((128, TOTC), np.float32)
        arr[:, :WCOLS] = wreg
        xs = xt_all[:, c * BPC : (c + 1) * BPC, :]  # [E, 64, SP]
        for g in range(NG):
            xb = xs[:, g * G : (g + 1) * G, :]  # [E, G, SP]
            # s-major planes: col n = s*G + b, so shift o = slice [4o:4o+512]
            xbT = xb.transpose(0, 2, 1).reshape(E, SPW)  # [E, 528]
            sh = np.stack(
                [xb[:, :, o : o + S].transpose(0, 2, 1).reshape(E, NMM)
                 for o in range(5)]
            )  # [5, E, 512]
            blk = np.zeros((128, 2, SPW + 3 * NMM), np.float32)
            blk[:, 0, :SPW] = xbT[0:128]
            blk[:, 1, :SPW] = xbT[128:256]
            for r, k in enumerate(KS):  # packed tails, shifts baked
                L = np.arange(E2N * k)
                i_of = L // E2N
                e_of = E2 + (L % E2N)
                off = (5 - k) + i_of
                blk[L // 2, L % 2, SPW + r * NMM : SPW + (r + 1) * NMM] = sh[
                    off, e_of
                ]
            base = WCOLS + g * GCOLS
            arr[:, base : base + 2 * SPW] = blk[:, :, :SPW].reshape(128, 2 * SPW)
            arr[:, base + 2 * SPW : base + GCOLS] = blk[:, :, SPW:].reshape(
                128, 6 * NMM
            )
        shards.append(arr.astype(ml_dtypes.float8_e4m3))
    return shards, auxm


def _run(inputs, **spmd_kwargs):
    global _built
    if _built is None:
        _built = _build()
    shards, auxm = _prep(**inputs)
    in_maps = [{"xq": shards[c], "aux": auxm} for c in range(NCORES)]
    res = run_bass_kernel_spmd(_built, in_maps, list(range(NCORES)), **spmd_kwargs)
    outp = np.concatenate(
        [np.asarray(res.results[c]["out"]).T for c in range(NCORES)], axis=0
    )
    return outp, res


def kernel(**inputs):
    outp, _ = _run(inputs)
    return outp


# revision 36
# speedup vs baseline: 1.0333x; 1.0333x over previous
"""Trainium2 Bass kernel: TextCNN (conv k=3/4/5 over [B,1,S,E] + relu +
global max-pool + FC + log_softmax), data-parallel over batch on 8 cores.

v2: conv contraction in fp8 e4m3 DoubleRow matmuls — each matmul
contracts 256 rows (two 128-row chunks paired along the DR j-dim), so a
group of 4 batch elems needs 15 matmuls instead of the fp32r version's
30, and fp8 streams at the PE's double rate. Weights are pre-scaled by
2^7 on the host to sit in e4m3's normal range; the scale is divided out
for free in the ReLU step via the activation unit's `scale` operand.
The E=300 contraction per tap splits as e[0:128]+e[128:256] paired in
one DR matmul (j = chunk index, identical column shift), plus a per-
branch host-baked tail matmul packing all taps' e[256:300] rows two-
per-partition. x for each 4-batch group lands in one [128, 4224B] DMA
(main pair block + 3 tail blocks); all 16 group DMAs are issued
upfront and stay resident in SBUF. FC + log_softmax stay fp32.

Every instruction carries <=1 semaphore wait (single wait slot in this
toolchain's TPB encodings): group g's x-DMA wait is pre-satisfied on
the last matmul of group g-1, so each branch-start matmul carries only
its PSUM-rotation wait. The kernel-tail drain is split per semaphore
proc.

Self-contained: hardcodes shapes/sharding; only imports the container
toolchain at /opt/trn_rl_repo.
"""

import sys

import ml_dtypes
import numpy as np

sys.path.insert(0, "/opt/trn_rl_repo")

import concourse.bass as bass  # noqa: E402
import concourse.tile as tile  # noqa: E402
from concourse import mybir  # noqa: E402
from concourse.bass_utils import run_bass_kernel_spmd  # noqa: E402
from concourse.tile import add_dep_helper  # noqa: E402
from concourse.vector_clock import ScopedClock, VectorClock  # noqa: E402

B, S, E = 512, 128, 300
NF = 100
NCLS = 5
NCORES = 8
BPC = B // NCORES  # 64 batch elems per core
G = 4  # batch elems per matmul group (4*128 = 512 psum cols)
NG = BPC // G  # 16 groups
PAD = 2
SP = S + 2 * PAD  # 132 padded seq length
KS = (3, 4, 5)
SOUT = {3: S - 2, 4: S - 1, 5: S}  # valid conv output positions per branch
SMM = S  # uniform matmul col window; invalid tail cols excluded by reduce
E2 = 256  # main contraction rows (two 128 chunks paired along DR j)
E2N = E - E2  # 44 tail rows per tap
WS = 128.0  # host weight scale into e4m3 range; divided out in ReLU
NFP = 112  # M padded to a 16B multiple (dual-fp8 LDW stride alignment)
# main taps in matmul order: (k, i, off); off = (5-k)+i is the xpad shift
MAINTAPS = [(k, i, 5 - k + i) for k in KS for i in range(k)]
NTAP = len(MAINTAPS)  # 12
NW = NTAP + 3  # + one packed tail per branch
WCOLS = NW * 2 * NFP  # 3360 fp8 weight cols
NMM = SMM * G  # 512 moving cols per matmul, s-major (n = s*G + b)
SPW = SP * G  # 528-col s-major main plane; shift o = slice [4o : 4o+512]
GCOLS = 2 * (SPW + 3 * NMM)  # 4128 fp8 x cols per group (j-major)
TOTC = WCOLS + NG * GCOLS
AUXW = 3 + 3 * NCLS
NPRE = 9  # prewarm matmuls bridging DMA ramp + PE clock ramp

_f32 = mybir.dt.float32
_fp8 = mybir.dt.float8e4
_DR = mybir.MatmulPerfMode.DoubleRow

_built = None


def _ins(i):
    return i.ins if hasattr(i, "ins") else i


def _dep(from_inst, to_inst, reason, sync=True):
    add_dep_helper(_ins(from_inst), _ins(to_inst), sync=sync, reason=reason)


class _SplitDrainTC(tile.TileContext):
    """TileContext whose kernel-tail drain is split into one drain per
    semaphore proc: the stock single drain carries one wait per used proc,
    which overflows the CTRL_NO encoding's wait slots on this toolchain."""

    def _drain_and_barrier(self, tick_clock, wait_clock):
        gc = tick_clock.global_clock
        ticks = eval(str(gc).replace("VectorClock", ""))
        for idx, tick in enumerate(ticks):
            if tick > 0:
                sub = VectorClock()
                sub.require_at_least(idx, tick)
                d = self.nc.sync.drain()
                wait_clock.add_sem_waits(d.ins, ScopedClock({None: sub}))
        self.nc.all_engine_barrier()
        assert self.sems is not None
        popped = self.nc._tile_sem_poison_stack.pop()
        assert popped is self._sem_poison
        self.nc.clear_and_free_semaphores(list(self.sems.allocated().values()))
        self.nc.all_engine_barrier()


def _build():
    nc = bass.Bass()
    xq = nc.declare_dram_parameter("xq", [128, TOTC], _fp8, isOutput=False)
    aux = nc.declare_dram_parameter("aux", [NF + 1, AUXW], _f32, isOutput=False)
    out = nc.declare_dram_parameter("out", [NCLS, BPC], _f32, isOutput=True)

    act = mybir.ActivationFunctionType

    with _SplitDrainTC(nc) as tc:
        with (
            tc.tile_pool(name="consts", bufs=1) as consts,
            tc.tile_pool(name="xin", bufs=NG) as xin,
            tc.tile_pool(name="small", bufs=4) as small,
            tc.tile_pool(name="feat", bufs=1) as featp,
            tc.tile_pool(name="psum", bufs=2, space="PSUM") as psum,
            tc.tile_pool(name="psfc", bufs=1, space="PSUM") as psfc,
        ):
            pescr = psfc.tile([128, 512], _f32, tag="pescr")
            junk = small.tile([128, 2, 512], _fp8, tag="junk")
            nc.vector.memset(junk[:], 0.25)

            # DMAs: weights first, then x groups (w, x0, x1, aux, x2..x15)
            wtile = consts.tile([128, NW, 2, NFP], _fp8, tag="w", name="w")
            wdma = nc.sync.dma_start(
                out=wtile[:],
                in_=xq[:, :WCOLS].rearrange("p (t j f) -> p t j f", t=NW, j=2),
            )

            xtiles = {}
            xmdmas = {}
            xtdmas = {}

            def make_xm(g, eng):
                t = xin.tile([128, 2, SPW + 3 * NMM], _fp8, tag="x", name=f"x_{g}")
                base = WCOLS + g * GCOLS
                # main plane first as its own small DMA: it unblocks the
                # group's main taps ~4x sooner than the full-group transfer
                # (the DMA engines run slow while ramping at kernel start)
                xmdmas[g] = eng.dma_start(
                    out=t[:, :, :SPW],
                    in_=xq[:, base : base + 2 * SPW].rearrange(
                        "p (j n) -> p j n", j=2
                    ),
                )
                xtiles[g] = t
                return t

            def make_xt(g, eng):
                t = xtiles[g]
                base = WCOLS + g * GCOLS
                xtdmas[g] = eng.dma_start(
                    out=t[:, :, SPW:],
                    in_=xq[:, base + 2 * SPW : base + GCOLS].rearrange(
                        "p (j n) -> p j n", j=2
                    ),
                )

            def make_x(g, eng):
                make_xm(g, eng)
                make_xt(g, eng)
                return xtiles[g]

            # two DMA rings share HBM bandwidth: alternate groups across
            # them in need-order so the earliest groups land first (a ring
            # prefetching late groups would starve the critical early ones);
            # the tiny aux slips between x0's main and tail transfers
            auxt = consts.tile([NF + 1, AUXW], _f32, tag="aux", name="aux")
            make_xm(0, nc.gpsimd)
            aux_dma = nc.gpsimd.dma_start(out=auxt[:], in_=aux[:, :])
            make_xt(0, nc.gpsimd)
            make_x(1, nc.sync)
            for g in range(2, NG):
                make_x(g, nc.gpsimd if g % 2 == 0 else nc.sync)

            # prewarm: fp8 DR junk matmuls bridge the DMA ramp so the PE
            # clock is up when the real stream starts; last one fences wdma.
            # sync=False edges pin the scheduler to this PE order — without
            # them it hoists later matmuls above the wait-carrying ones and
            # the single-wait-slot budget breaks.
            last_pe = None
            for p in range(NPRE):
                pw = nc.tensor.matmul(
                    pescr[:, :],
                    junk[:, :, :128],
                    junk[:, :, :],
                    start=True,
                    stop=True,
                    perf_mode=_DR,
                )
                if last_pe is not None:
                    _dep(pw, last_pe, "pe chain", sync=False)
                last_pe = pw
                if p == NPRE - 1:
                    _dep(pw, wdma, "w loaded")

            ascratch = small.tile([1, 1], _f32, tag="ascratch")
            feats = [
                featp.tile([NF, BPC], _f32, tag=f"feat{kk}", name=f"feat{kk}")
                for kk in range(3)
            ]
            featr = [
                featp.tile(
                    [NF + (1 if kk == 2 else 0), BPC],
                    _f32,
                    tag=f"featr{kk}",
                    name=f"featr{kk}",
                )
                for kk in range(3)
            ]
            nc.vector.memset(featr[2][:], 1.0)

            plT = psfc.tile([NCLS, BPC], _f32, tag="plT")
            ones5 = small.tile([NCLS, 1], _f32, tag="ones5")
            nc.vector.memset(ones5[:], 1.0)
            mones1 = small.tile([1, NCLS], _f32, tag="mones1")
            nc.vector.memset(mones1[:], -1.0)
            afence = nc.scalar.memzero(ascratch[:])
            _dep(afence, aux_dma, "act waits aux")
            # touch Exp/Ln tables now so the tail doesn't pay cold loads
            nc.scalar.activation(ascratch[:], ascratch[:], act.Exp)
            nc.scalar.activation(ascratch[:], ascratch[:], act.Ln)

            gmms = {}  # group -> list of its 15 conv matmuls
            greds = {}  # group -> last reduce_max
            for g in range(NG):
                xt = xtiles[g]
                if g >= 1:
                    # pre-satisfy group g's cross-queue waits on spare
                    # (waitless) matmuls of group g-1, so this group's
                    # branch-start matmuls carry no >1-wait encodings:
                    #   x DMAs done, g-2's reduces done (frees PSUM banks,
                    #   DVE sem), g-2's stop-matmul completed (PE sem).
                    prev = gmms[g - 1]
                    _dep(prev[4], xmdmas[g], "x main presat")
                    _dep(prev[5], xtdmas[g], "x tail presat")
                    if g >= 2:
                        _dep(prev[8], greds[g - 2], "psum reduce presat")
                        _dep(prev[9], gmms[g - 2][14], "psum group presat")

                ti = 0
                mms = []

                def _mm(*args, **kw):
                    nonlocal last_pe
                    m = nc.tensor.matmul(*args, **kw)
                    _dep(m, last_pe, "pe chain", sync=False)
                    last_pe = m
                    mms.append(m)
                    return m

                def _fc(s, last):
                    for kk in range(3):
                        krows = NF + (1 if kk == 2 else 0)
                        wsl = auxt[:krows, 3 + NCLS * kk : 3 + NCLS * (kk + 1)]
                        fc = _mm(
                            plT[:, 16 * s : 16 * (s + 1)],
                            wsl,
                            featr[kk][:krows, 16 * s : 16 * (s + 1)],
                            start=(s == 0 and kk == 0),
                            stop=False,
                        )
                    return fc

                # FC block s runs two groups after its features are done, so
                # its Scalar-side relu wait is long satisfied (no PE stall)
                if g >= 5 and (g - 5) % 4 == 0:
                    _fc((g - 5) // 4, last=False)

                # main taps of all branches first (they need only the small
                # main-plane DMA), then the three tails — so the early
                # groups start well before their tail transfer lands. The
                # LAST group instead runs branch-sequential, so k3/k4's
                # reduce+relu+FC overlap the remaining conv matmuls instead
                # of serializing on the DVE after the final matmul.
                seq = g == NG - 1

                def _tail(kk, k, ps):
                    _mm(
                        ps[:, :, :],
                        wtile[:, NTAP + kk, :, :],
                        xt[:, :, SPW + kk * NMM : SPW + (kk + 1) * NMM],
                        start=False,
                        stop=True,
                        perf_mode=_DR,
                    )
                    red = nc.vector.reduce_max(
                        feats[kk][:, g * G : (g + 1) * G],
                        ps[:NF, : SOUT[k], :].transpose([0, 2, 1]),
                        axis=mybir.AxisListType.X,
                    )
                    r = nc.scalar.activation(
                        featr[kk][:NF, g * G : (g + 1) * G],
                        feats[kk][:, g * G : (g + 1) * G],
                        act.Relu,
                        bias=auxt[:NF, kk : kk + 1],
                        scale=1.0 / WS,
                    )
                    _dep(r, afence, "act fence", sync=False)
                    return red

                pss = {}
                for kk, k in enumerate(KS):
                    ps = psum.tile([NFP, SMM, G], _f32, tag=f"y{k}", name=f"y{k}_{g}")
                    pss[kk] = ps
                    for i in range(k):
                        off = 5 - k + i
                        _mm(
                            ps[:, :, :],
                            wtile[:, ti, :, :],
                            xt[:, :, 4 * off : 4 * off + NMM],
                            start=(i == 0),
                            stop=False,
                            perf_mode=_DR,
                        )
                        ti += 1
                    if seq:
                        red = _tail(kk, k, ps)
                if g == 2:
                    _dep(mms[1], aux_dma, "aux presat for FC")
                if not seq:
                    for kk, k in enumerate(KS):
                        red = _tail(kk, k, pss[kk])
                # conv matmuls only (indices 0..14) — FC mms are appended
                # after, so the carrier indices stay conv taps
                gmms[g] = mms[-15:]
                greds[g] = red

            _fc(3, last=True)

            # log_softmax in transposed layout: out = x - ln(sum exp x); the
            # class-dim sum is a tiny ones-matmul, the broadcast subtract a
            # single DVE op reading plT (PSUM) directly
            expT = small.tile([NCLS, BPC], _f32, tag="expT")
            nc.scalar.activation(expT[:], plT[:], act.Exp)
            nc.tensor.matmul(
                pescr[0:1, 64:128], ones5[:], expT[:], start=True, stop=True
            )
            lns = small.tile([1, BPC], _f32, tag="lns")
            nc.scalar.activation(lns[:], pescr[0:1, 64:128], act.Ln)
            nc.tensor.matmul(plT[:], mones1[:], lns[:], start=False, stop=True)
            # copy + output DMA both on the Scalar queue: the DMA then needs
            # no data wait (queue order) — only its hardware-ring slot
            ot = small.tile([NCLS, BPC], _f32, tag="ot")
            cp = nc.scalar.activation(ot[:], plT[:], act.Copy)
            oscr = small.tile([1, 1], _f32, tag="oscr")
            f2 = nc.scalar.memzero(oscr[:])
            _dep(f2, cp, "ot ready")
            nc.scalar.dma_start(out=out[:, :], in_=ot[:])
    return nc


def _prep(x, w3, b3, w4, b4, w5, b5, Wfc, bfc):
    x = np.asarray(x, dtype=np.float32).reshape(B, S, E)
    ws = {3: np.asarray(w3, np.float32)[:, 0], 4: np.asarray(w4, np.float32)[:, 0],
          5: np.asarray(w5, np.float32)[:, 0]}  # [NF, k, E]

    # weights region (identical across cores), assembled fp32 then cast once
    wreg = np.zeros((128, NW, 2, NFP), np.float32)
    for t, (k, i, _off) in enumerate(MAINTAPS):
        for j in range(2):
            wreg[:, t, j, :NF] = WS * ws[k][:, i, j * 128 : (j + 1) * 128].T
    for r, k in enumerate(KS):
        L = np.arange(E2N * k)
        i_of = L // E2N
        e_of = E2 + (L % E2N)
        wt = ws[k][:, i_of, e_of].T * WS  # [L, NF]
        wreg[L // 2, NTAP + r, L % 2, :NF] = wt
    wreg = wreg.reshape(128, WCOLS)

    # x padded + transposed: [E, B, SP]
    xt_all = np.zeros((E, B, SP), np.float32)
    xt_all[:, :, PAD : PAD + S] = x.transpose(2, 0, 1)

    auxm = np.zeros((NF + 1, AUXW), np.float32)
    for kk, bb in enumerate((b3, b4, b5)):
        auxm[:NF, kk] = np.asarray(bb, np.float32)
    Wfc = np.asarray(Wfc, np.float32)
    for kk in range(3):
        auxm[:NF, 3 + NCLS * kk : 3 + NCLS * (kk + 1)] = Wfc[
            :, kk * NF : (kk + 1) * NF
        ].T
    auxm[NF, 3 + 2 * NCLS : 3 + 3 * NCLS] = np.asarray(bfc, np.float32)

    shards = []
    for c in range(NCORES):
        arr = np.zeros((128, TOTC), np.float32)
        arr[:, :WCOLS] = wreg
        xs = xt_all[:, c * BPC : (c + 1) * BPC, :]  # [E, 64, SP]
        for g in range(NG):
            xb = xs[:, g * G : (g + 1) * G, :]  # [E, G, SP]
            # s-major planes: col n = s*G + b, so shift o = slice [4o:4o+512]
            xbT = xb.transpose(0, 2, 1).reshape(E, SPW)  # [E, 528]
            sh = np.stack(
                [xb[:, :, o : o + S].transpose(0, 2, 1).reshape(E, NMM)
                 for o in range(5)]
            )  # [5, E, 512]
            blk = np.zeros((128, 2, SPW + 3 * NMM), np.float32)
            blk[:, 0, :SPW] = xbT[0:128]
            blk[:, 1, :SPW] = xbT[128:256]
            for r, k in enumerate(KS):  # packed tails, shifts baked
                L = np.arange(E2N * k)
                i_of = L // E2N
                e_of = E2 + (L % E2N)
                off = (5 - k) + i_of
                blk[L // 2, L % 2, SPW + r * NMM : SPW + (r + 1) * NMM] = sh[
                    off, e_of
                ]
            base = WCOLS + g * GCOLS
            arr[:, base : base + 2 * SPW] = blk[:, :, :SPW].reshape(128, 2 * SPW)
            arr[:, base + 2 * SPW : base + GCOLS] = blk[:, :, SPW:].reshape(
                128, 6 * NMM
            )
        shards.append(arr.astype(ml_dtypes.float8_e4m3))
    return shards, auxm


def _run(inputs, **spmd_kwargs):
    global _built
    if _built is None:
        _built = _build()
    shards, auxm = _prep(**inputs)
    in_maps = [{"xq": shards[c], "aux": auxm} for c in range(NCORES)]
    res = run_bass_kernel_spmd(_built, in_maps, list(range(NCORES)), **spmd_kwargs)
    outp = np.concatenate(
        [np.asarray(res.results[c]["out"]).T for c in range(NCORES)], axis=0
    )
    return outp, res


def kernel(**inputs):
    outp, _ = _run(inputs)
    return outp
